# revision 14
# baseline (speedup 1.0000x reference)
"""Trainium2 Bass kernel for ColumnConsistencyLoss (segment_reduce).

Problem: B=16, T=8192, C=128.
  probs = softmax(logits, -1)           # (N, C), N = B*T = 131072
  per column-id c (segment): n_c = #valid tokens, S_c = sum w*p, Q_c = sum w*p^2
  col_var_c = (sum_j Q_cj - sum_j S_cj^2 / n_safe_c) / (n_safe_c * C)
  loss = mean over columns with n_c > 1 of col_var_c

Sharding: data-parallel over tokens — each of the 8 cores processes
N/8 = 16384 tokens and produces partial per-segment accumulators
S (C x C) and Q (C x C).  The cross-core reduction of these tiny
accumulators plus the final scalar math happens on the host (exact
counts n_c are computed on host via bincount).

Device kernel per core (v3 — single streaming sweep, tokens on partitions):
  - host precomputes M = onehot(seg) * w  (fp8, exact 0/1 values)
  - DMA logits in non-uniform chunks [8,16,32,32,32,8] tiles (small first
    chunk fills the pipeline early; small last chunk shortens the tail)
  - ScalarE: E = exp(L) -> bf16 (big-FD chunk)
  - DVE:     d = rowsum(E) via 2x halving adds (bf16 2x mode) + reduce
  - DVE:     r ~= 1/d (reciprocal_approx_fast, ~51 ULP)
  - DVE:     rhs[:, :, 0, :] = E * broadcast(r)   (one chunk-level TT)
  - ScalarE: rhs[:, :, 1, :] = Square(rhs[:, :, 0, :])  (2 half-chunk ACTs)
  - PE:      psum[(c),(s,j')] += M_j^T @ rhs[:,j,:,:]  (F=256, fp32 accum)
The matmul contracts the 128 partitions (tokens); w rides in M:
  psum[c,0,:] = sum_t w*1[seg=c] * (E/d)        = S_c
  psum[c,1,:] = sum_t w*1[seg=c] * (E/d)^2      = Q_c   (w^2 = w).
"""

import numpy as np
import ml_dtypes

NCORES = 8
P = 128           # partitions
C = 128           # columns / segments
B, T = 16, 8192
N_TOK = B * T
TOK_PER_CORE = N_TOK // NCORES   # 16384
J_FULL = TOK_PER_CORE // P       # 128 free-columns (token tiles) per core
CHUNKS = (4, 12, 28, 28, 28, 28)  # token tiles per DMA/compute chunk

TRACE = False          # set True (e.g. from test.py) to capture NTFF profile
TRACE_TMPDIR = None    # where trace/NEFF artifacts land when TRACE is set
LAST_RESULT = None     # BassKernelResults of the last run (for profiling)

_NC_CACHE = {}


def build_nc(chunks=CHUNKS):
    """Build + compile the Bass program (SPMD; same NEFF on all cores)."""
    from concourse import bacc, mybir
    import concourse.tile as tile

    f32 = mybir.dt.float32
    bf16 = mybir.dt.bfloat16
    fp8 = mybir.dt.float8e4
    Exp = mybir.ActivationFunctionType.Exp
    Square = mybir.ActivationFunctionType.Square
    Alu = mybir.AluOpType

    j_full = sum(chunks)
    tok = j_full * P
    H = C // 2   # 64
    Q4 = C // 4  # 32

    nc = bacc.Bacc("TRN2", target_bir_lowering=False, debug=False,
                   enable_asserts=False)

    lg_d = nc.dram_tensor("logits", [tok, C], f32, kind="ExternalInput")
    m_d = nc.dram_tensor("m8", [tok, C], fp8, kind="ExternalInput")
    sq_d = nc.dram_tensor("sq_out", [C, 2, C], f32, kind="ExternalOutput")

    with tile.TileContext(nc) as tc:
        with (
            tc.tile_pool(name="const", bufs=1) as constp,
            tc.tile_pool(name="ld", bufs=4) as ldp,
            tc.tile_pool(name="big", bufs=3) as bigp,
            tc.tile_pool(name="small", bufs=4) as smallp,
            tc.tile_pool(name="psum", bufs=1, space="PSUM") as psump,
        ):
            psum_sq = psump.tile([C, 2, C], f32)

            # DRAM views: (p, j, c) with token t = p*j_full + j
            lg_ap = lg_d[:].rearrange("(p j) c -> p j c", j=j_full)
            m_ap = m_d[:].rearrange("(p j) c -> p j c", j=j_full)

            js = 0
            for k, cj in enumerate(chunks):
                L = ldp.tile([P, cj, C], f32, tag="L")
                nc.sync.dma_start(L[:], lg_ap[:, js:js + cj, :])
                M8 = ldp.tile([P, cj, C], fp8, tag="M8")
                # scalar-issued DMA rides the second HWDGE ring, so the
                # one-hot load doesn't queue behind the logits stream
                nc.scalar.dma_start(M8[:], m_ap[:, js:js + cj, :])

                E = bigp.tile([P, cj, C], bf16, tag="E")
                nc.scalar.activation(E[:], L[:], Exp)

                # d = rowsum(E): two bf16 2x halving adds + 1x reduce
                h1 = bigp.tile([P, cj, H], bf16, tag="h1")
                nc.vector.tensor_tensor(h1[:], E[:, :, 0:H], E[:, :, H:C],
                                        op=Alu.add)
                h2 = bigp.tile([P, cj, Q4], bf16, tag="h2")
                nc.vector.tensor_tensor(h2[:], h1[:, :, 0:Q4], h1[:, :, Q4:H],
                                        op=Alu.add)
                d = smallp.tile([P, cj], f32, tag="d")
                nc.vector.tensor_reduce(d[:], h2[:], axis=mybir.AxisListType.X,
                                        op=Alu.add)
                r = smallp.tile([P, cj], f32, tag="r")
                nc.vector.reciprocal_approx_fast(r[:], d[:])

                rhs = bigp.tile([P, cj, 2, C], bf16, tag="rhs")
                # normalized probs: one chunk-level TT with broadcast in1
                nc.vector.tensor_tensor(
                    rhs[:, :, 0, :], E[:],
                    r[:, :, None].to_broadcast([P, cj, C]),
                    op=Alu.mult)
                # squared half; split in two ACTs on the last chunk only so
                # the tail matmuls start earlier
                if k == len(chunks) - 1 and cj > 1:
                    hh = cj // 2
                    nc.scalar.activation(rhs[:, 0:hh, 1, :],
                                         rhs[:, 0:hh, 0, :], Square)
                    nc.scalar.activation(rhs[:, hh:cj, 1, :],
                                         rhs[:, hh:cj, 0, :], Square)
                else:
                    nc.scalar.activation(rhs[:, :, 1, :], rhs[:, :, 0, :],
                                         Square)

                for jj in range(cj):
                    j = js + jj
                    nc.tensor.matmul(
                        psum_sq[:], M8[:, jj, :], rhs[:, jj, :, :],
                        start=(j == 0), stop=(j == j_full - 1))
                js += cj

            out_t = constp.tile([C, 2, C], f32)
            nc.vector.tensor_copy(out_t[:], psum_sq[:])
            nc.sync.dma_start(sq_d[:], out_t[:])

    nc.compile()
    return nc


def _get_nc():
    key = CHUNKS
    if key not in _NC_CACHE:
        _NC_CACHE[key] = build_nc(key)
    return _NC_CACHE[key]


def kernel(column_logits, column_assignments, valid_mask):
    global LAST_RESULT
    from concourse.bass_utils import run_bass_kernel_spmd

    logits = np.asarray(column_logits, dtype=np.float32).reshape(N_TOK, C)
    seg = np.asarray(column_assignments).reshape(N_TOK).astype(np.int64)
    w = np.asarray(valid_mask).reshape(N_TOK).astype(bool)

    fp8np = ml_dtypes.float8_e4m3
    M8_full = np.zeros((N_TOK, C), dtype=fp8np)
    M8_full[np.arange(N_TOK)[w], seg[w]] = fp8np(1.0)   # w folded into M

    in_maps = []
    for i in range(NCORES):
        sl = slice(i * TOK_PER_CORE, (i + 1) * TOK_PER_CORE)
        in_maps.append({
            "logits": np.ascontiguousarray(logits[sl]),
            "m8": np.ascontiguousarray(M8_full[sl]),
        })

    nc = _get_nc()
    res = run_bass_kernel_spmd(nc, in_maps, list(range(NCORES)), trace=TRACE,
                               tmpdir=TRACE_TMPDIR)
    LAST_RESULT = res

    SQ = np.zeros((C, 2, C), np.float64)
    for rm in res.results:
        SQ += np.asarray(rm["sq_out"], dtype=np.float64)
    S = SQ[:, 0, :]
    Q = SQ[:, 1, :]

    n = np.bincount(seg[w], minlength=C).astype(np.float64)
    n_safe = np.maximum(n, 1.0)
    ssd_sum = Q.sum(axis=1) - (S * S).sum(axis=1) / n_safe
    col_var = ssd_sum / (n_safe * C)
    has_multi = n > 1.0
    count = has_multi.sum()
    total = np.where(has_multi, col_var, 0.0).sum()
    loss = total / max(count, 1.0) if count > 0 else 0.0
    return np.asarray(loss, dtype=np.float32)


# revision 16
# speedup vs baseline: 1.1239x; 1.1239x over previous
"""Trainium2 Bass kernel for ColumnConsistencyLoss (segment_reduce).

Problem: B=16, T=8192, C=128.
  probs = softmax(logits, -1)           # (N, C), N = B*T = 131072
  per column-id c (segment): n_c = #valid tokens, S_c = sum w*p, Q_c = sum w*p^2
  col_var_c = (sum_j Q_cj - sum_j S_cj^2 / n_safe_c) / (n_safe_c * C)
  loss = mean over columns with n_c > 1 of col_var_c

Sharding: data-parallel over tokens — each of the 8 cores processes
N/8 = 16384 tokens and produces partial per-segment accumulators
S (C x C) and Q (C x C).  The cross-core reduction of these tiny
accumulators plus the final scalar math happens on the host (exact
counts n_c are computed on host via bincount).

Device kernel per core (v3 — single streaming sweep, tokens on partitions):
  - host precomputes M = onehot(seg) * w  (fp8, exact 0/1 values)
  - DMA logits in non-uniform chunks [8,16,32,32,32,8] tiles (small first
    chunk fills the pipeline early; small last chunk shortens the tail)
  - ScalarE: E = exp(L) -> bf16 (big-FD chunk)
  - DVE:     d = rowsum(E) via 2x halving adds (bf16 2x mode) + reduce
  - DVE:     r ~= 1/d (reciprocal_approx_fast, ~51 ULP)
  - DVE:     rhs[:, :, 0, :] = E * broadcast(r)   (one chunk-level TT)
  - ScalarE: rhs[:, :, 1, :] = Square(rhs[:, :, 0, :])  (2 half-chunk ACTs)
  - PE:      psum[(c),(s,j')] += M_j^T @ rhs[:,j,:,:]  (F=256, fp32 accum)
The matmul contracts the 128 partitions (tokens); w rides in M:
  psum[c,0,:] = sum_t w*1[seg=c] * (E/d)        = S_c
  psum[c,1,:] = sum_t w*1[seg=c] * (E/d)^2      = Q_c   (w^2 = w).
"""

import numpy as np
import ml_dtypes

NCORES = 8
P = 128           # partitions
C = 128           # columns / segments
B, T = 16, 8192
N_TOK = B * T
TOK_PER_CORE = N_TOK // NCORES   # 16384
J_FULL = TOK_PER_CORE // P       # 128 free-columns (token tiles) per core
CHUNKS = (4, 12, 24, 28, 28, 24, 8)  # token tiles per DMA/compute chunk

TRACE = False          # set True (e.g. from test.py) to capture NTFF profile
TRACE_TMPDIR = None    # where trace/NEFF artifacts land when TRACE is set
LAST_RESULT = None     # BassKernelResults of the last run (for profiling)

_NC_CACHE = {}


def build_nc(chunks=CHUNKS):
    """Build + compile the Bass program (SPMD; same NEFF on all cores)."""
    from concourse import bacc, mybir
    import concourse.tile as tile

    f32 = mybir.dt.float32
    bf16 = mybir.dt.bfloat16
    fp8 = mybir.dt.float8e4
    Exp = mybir.ActivationFunctionType.Exp
    Square = mybir.ActivationFunctionType.Square
    Alu = mybir.AluOpType

    j_full = sum(chunks)
    tok = j_full * P
    H = C // 2   # 64
    Q4 = C // 4  # 32

    nc = bacc.Bacc("TRN2", target_bir_lowering=False, debug=False,
                   enable_asserts=False)

    lg_d = nc.dram_tensor("logits", [tok, C], f32, kind="ExternalInput")
    m_d = nc.dram_tensor("m8", [tok, C], fp8, kind="ExternalInput")
    sq_d = nc.dram_tensor("sq_out", [C, 2, C], f32, kind="ExternalOutput")

    with tile.TileContext(nc) as tc:
        with (
            tc.tile_pool(name="const", bufs=1) as constp,
            tc.tile_pool(name="ld", bufs=4) as ldp,
            tc.tile_pool(name="big", bufs=3) as bigp,
            tc.tile_pool(name="small", bufs=4) as smallp,
            tc.tile_pool(name="psum", bufs=1, space="PSUM") as psump,
        ):
            psum_sq = psump.tile([C, 2, C], f32)

            # DRAM views: (p, j, c) with token t = p*j_full + j
            lg_ap = lg_d[:].rearrange("(p j) c -> p j c", j=j_full)
            m_ap = m_d[:].rearrange("(p j) c -> p j c", j=j_full)

            nchunk = len(chunks)
            offs = [sum(chunks[:k]) for k in range(nchunk)]
            Ls = [None] * nchunk
            Ms = [None] * nchunk
            Es = [None] * nchunk

            def emit_load(k):
                cj = chunks[k]
                L = ldp.tile([P, cj, C], f32, tag="L")
                nc.sync.dma_start(L[:], lg_ap[:, offs[k]:offs[k] + cj, :])
                M8 = ldp.tile([P, cj, C], fp8, tag="M8")
                # scalar-issued DMA rides the second HWDGE ring, so the
                # one-hot load doesn't queue behind the logits stream
                nc.scalar.dma_start(M8[:], m_ap[:, offs[k]:offs[k] + cj, :])
                Ls[k], Ms[k] = L, M8

            def emit_exp(k):
                cj = chunks[k]
                E = bigp.tile([P, cj, C], bf16, tag="E")
                nc.scalar.activation(E[:], Ls[k][:], Exp)
                Es[k] = E

            emit_load(0)
            emit_load(1)
            emit_exp(0)
            for k, cj in enumerate(chunks):
                if k + 2 < nchunk:
                    emit_load(k + 2)
                E, M8, js = Es[k], Ms[k], offs[k]

                # d = rowsum(E): two bf16 2x halving adds + 1x reduce
                h1 = bigp.tile([P, cj, H], bf16, tag="h1")
                nc.vector.tensor_tensor(h1[:], E[:, :, 0:H], E[:, :, H:C],
                                        op=Alu.add)
                h2 = bigp.tile([P, cj, Q4], bf16, tag="h2")
                nc.vector.tensor_tensor(h2[:], h1[:, :, 0:Q4], h1[:, :, Q4:H],
                                        op=Alu.add)
                d = smallp.tile([P, cj], f32, tag="d")
                nc.vector.tensor_reduce(d[:], h2[:], axis=mybir.AxisListType.X,
                                        op=Alu.add)
                r = smallp.tile([P, cj], f32, tag="r")
                nc.vector.reciprocal_approx_fast(r[:], d[:])

                rhs = bigp.tile([P, cj, 2, C], bf16, tag="rhs")
                # normalized probs: one chunk-level TT with broadcast in1
                nc.vector.tensor_tensor(
                    rhs[:, :, 0, :], E[:],
                    r[:, :, None].to_broadcast([P, cj, C]),
                    op=Alu.mult)

                # next chunk's exp BEFORE this chunk's squares, so the DVE
                # chain of chunk k+1 is never starved behind scalar work
                if k + 1 < nchunk:
                    emit_exp(k + 1)

                # squared half in two ACTs (finer scalar interleave)
                hh = max(cj // 2, 1)
                nc.scalar.activation(rhs[:, 0:hh, 1, :], rhs[:, 0:hh, 0, :],
                                     Square)
                if hh < cj:
                    nc.scalar.activation(rhs[:, hh:cj, 1, :],
                                         rhs[:, hh:cj, 0, :], Square)

                for jj in range(cj):
                    j = js + jj
                    nc.tensor.matmul(
                        psum_sq[:], M8[:, jj, :], rhs[:, jj, :, :],
                        start=(j == 0), stop=(j == j_full - 1))

            out_t = constp.tile([C, 2, C], f32)
            nc.vector.tensor_copy(out_t[:], psum_sq[:])
            nc.sync.dma_start(sq_d[:], out_t[:])

    nc.compile()
    return nc


def _get_nc():
    key = CHUNKS
    if key not in _NC_CACHE:
        _NC_CACHE[key] = build_nc(key)
    return _NC_CACHE[key]


def kernel(column_logits, column_assignments, valid_mask):
    global LAST_RESULT
    from concourse.bass_utils import run_bass_kernel_spmd

    logits = np.asarray(column_logits, dtype=np.float32).reshape(N_TOK, C)
    seg = np.asarray(column_assignments).reshape(N_TOK).astype(np.int64)
    w = np.asarray(valid_mask).reshape(N_TOK).astype(bool)

    fp8np = ml_dtypes.float8_e4m3
    M8_full = np.zeros((N_TOK, C), dtype=fp8np)
    M8_full[np.arange(N_TOK)[w], seg[w]] = fp8np(1.0)   # w folded into M

    in_maps = []
    for i in range(NCORES):
        sl = slice(i * TOK_PER_CORE, (i + 1) * TOK_PER_CORE)
        in_maps.append({
            "logits": np.ascontiguousarray(logits[sl]),
            "m8": np.ascontiguousarray(M8_full[sl]),
        })

    nc = _get_nc()
    res = run_bass_kernel_spmd(nc, in_maps, list(range(NCORES)), trace=TRACE,
                               tmpdir=TRACE_TMPDIR)
    LAST_RESULT = res

    SQ = np.zeros((C, 2, C), np.float64)
    for rm in res.results:
        SQ += np.asarray(rm["sq_out"], dtype=np.float64)
    S = SQ[:, 0, :]
    Q = SQ[:, 1, :]

    n = np.bincount(seg[w], minlength=C).astype(np.float64)
    n_safe = np.maximum(n, 1.0)
    ssd_sum = Q.sum(axis=1) - (S * S).sum(axis=1) / n_safe
    col_var = ssd_sum / (n_safe * C)
    has_multi = n > 1.0
    count = has_multi.sum()
    total = np.where(has_multi, col_var, 0.0).sum()
    loss = total / max(count, 1.0) if count > 0 else 0.0
    return np.asarray(loss, dtype=np.float32)


# revision 18
# speedup vs baseline: 1.1472x; 1.0208x over previous
"""Trainium2 Bass kernel for ColumnConsistencyLoss (segment_reduce).

Problem: B=16, T=8192, C=128.
  probs = softmax(logits, -1)           # (N, C), N = B*T = 131072
  per column-id c (segment): n_c = #valid tokens, S_c = sum w*p, Q_c = sum w*p^2
  col_var_c = (sum_j Q_cj - sum_j S_cj^2 / n_safe_c) / (n_safe_c * C)
  loss = mean over columns with n_c > 1 of col_var_c

Sharding: data-parallel over tokens — each of the 8 cores processes
N/8 = 16384 tokens and produces partial per-segment accumulators
S (C x C) and Q (C x C).  The cross-core reduction of these tiny
accumulators plus the final scalar math happens on the host (exact
counts n_c are computed on host via bincount).

Device kernel per core (v3 — single streaming sweep, tokens on partitions):
  - host precomputes M = onehot(seg) * w  (fp8, exact 0/1 values)
  - DMA logits in non-uniform chunks [8,16,32,32,32,8] tiles (small first
    chunk fills the pipeline early; small last chunk shortens the tail)
  - ScalarE: E = exp(L) -> bf16 (big-FD chunk)
  - DVE:     d = rowsum(E) via 2x halving adds (bf16 2x mode) + reduce
  - DVE:     r ~= 1/d (reciprocal_approx_fast, ~51 ULP)
  - DVE:     rhs[:, :, 0, :] = E * broadcast(r)   (one chunk-level TT)
  - ScalarE: rhs[:, :, 1, :] = Square(rhs[:, :, 0, :])  (2 half-chunk ACTs)
  - PE:      psum[(c),(s,j')] += M_j^T @ rhs[:,j,:,:]  (F=256, fp32 accum)
The matmul contracts the 128 partitions (tokens); w rides in M:
  psum[c,0,:] = sum_t w*1[seg=c] * (E/d)        = S_c
  psum[c,1,:] = sum_t w*1[seg=c] * (E/d)^2      = Q_c   (w^2 = w).
"""

import numpy as np
import ml_dtypes

NCORES = 8
P = 128           # partitions
C = 128           # columns / segments
B, T = 16, 8192
N_TOK = B * T
TOK_PER_CORE = N_TOK // NCORES   # 16384
J_FULL = TOK_PER_CORE // P       # 128 free-columns (token tiles) per core
CHUNKS = (4, 12, 24, 28, 28, 24, 8)  # token tiles per DMA/compute chunk

TRACE = False          # set True (e.g. from test.py) to capture NTFF profile
TRACE_TMPDIR = None    # where trace/NEFF artifacts land when TRACE is set
LAST_RESULT = None     # BassKernelResults of the last run (for profiling)

_NC_CACHE = {}


def build_nc(chunks=CHUNKS):
    """Build + compile the Bass program (SPMD; same NEFF on all cores)."""
    from concourse import bacc, mybir
    import concourse.tile as tile

    f32 = mybir.dt.float32
    bf16 = mybir.dt.bfloat16
    fp8 = mybir.dt.float8e4
    Exp = mybir.ActivationFunctionType.Exp
    Square = mybir.ActivationFunctionType.Square
    Alu = mybir.AluOpType

    j_full = sum(chunks)
    tok = j_full * P
    H = C // 2   # 64
    Q4 = C // 4  # 32

    nc = bacc.Bacc("TRN2", target_bir_lowering=False, debug=False,
                   enable_asserts=False)

    lg_d = nc.dram_tensor("logits", [tok, C], f32, kind="ExternalInput")
    m_d = nc.dram_tensor("m8", [tok, C], fp8, kind="ExternalInput")
    sq_d = nc.dram_tensor("sq_out", [C, 2, C], f32, kind="ExternalOutput")

    with tile.TileContext(nc) as tc:
        with (
            tc.tile_pool(name="const", bufs=1) as constp,
            tc.tile_pool(name="ld", bufs=4) as ldp,
            tc.tile_pool(name="big", bufs=3) as bigp,
            tc.tile_pool(name="small", bufs=4) as smallp,
            tc.tile_pool(name="psum", bufs=1, space="PSUM") as psump,
        ):
            psum_sq = psump.tile([C, 2, C], f32)

            # DRAM views: (p, j, c) with token t = p*j_full + j
            lg_ap = lg_d[:].rearrange("(p j) c -> p j c", j=j_full)
            m_ap = m_d[:].rearrange("(p j) c -> p j c", j=j_full)

            nchunk = len(chunks)
            offs = [sum(chunks[:k]) for k in range(nchunk)]
            Ls = [None] * nchunk
            Ms = [None] * nchunk
            Es = [None] * nchunk

            def emit_load(k):
                cj = chunks[k]
                L = ldp.tile([P, cj, C], f32, tag="L")
                nc.sync.dma_start(L[:], lg_ap[:, offs[k]:offs[k] + cj, :])
                M8 = ldp.tile([P, cj, C], fp8, tag="M8")
                # scalar-issued DMA rides the second HWDGE ring, so the
                # one-hot load doesn't queue behind the logits stream
                nc.scalar.dma_start(M8[:], m_ap[:, offs[k]:offs[k] + cj, :])
                Ls[k], Ms[k] = L, M8

            def halves(cj):
                # sub-ranges to process at half-chunk granularity
                if cj >= 16:
                    return [(0, cj // 2), (cj // 2, cj)]
                return [(0, cj)]

            def emit_exp(k):
                cj = chunks[k]
                E = bigp.tile([P, cj, C], bf16, tag="E")
                for a, b in halves(cj):
                    nc.scalar.activation(E[:, a:b, :], Ls[k][:, a:b, :], Exp)
                Es[k] = E

            emit_load(0)
            emit_load(1)
            emit_exp(0)
            for k, cj in enumerate(chunks):
                if k + 2 < nchunk:
                    emit_load(k + 2)
                E, M8, js = Es[k], Ms[k], offs[k]
                rhs = bigp.tile([P, cj, 2, C], bf16, tag="rhs")
                sub = halves(cj)

                h1 = bigp.tile([P, cj, H], bf16, tag="h1")
                h2 = bigp.tile([P, cj, Q4], bf16, tag="h2")
                d = smallp.tile([P, cj], f32, tag="d")
                r = smallp.tile([P, cj], f32, tag="r")
                for si, (a, b) in enumerate(sub):
                    w = b - a
                    # d = rowsum(E): two bf16 2x halving adds + 1x reduce
                    nc.vector.tensor_tensor(h1[:, a:b, :], E[:, a:b, 0:H],
                                            E[:, a:b, H:C], op=Alu.add)
                    nc.vector.tensor_tensor(h2[:, a:b, :], h1[:, a:b, 0:Q4],
                                            h1[:, a:b, Q4:H], op=Alu.add)
                    nc.vector.tensor_reduce(d[:, a:b], h2[:, a:b, :],
                                            axis=mybir.AxisListType.X,
                                            op=Alu.add)
                    nc.vector.reciprocal_approx_fast(r[:, a:b], d[:, a:b])
                    # normalized probs: TT with broadcast in1
                    nc.vector.tensor_tensor(
                        rhs[:, a:b, 0, :], E[:, a:b, :],
                        r[:, a:b, None].to_broadcast([P, w, C]),
                        op=Alu.mult)
                    # next chunk's exp between this chunk's halves so the
                    # next DVE chain is never starved behind scalar work
                    if si == 0 and k + 1 < nchunk:
                        emit_exp(k + 1)
                    nc.scalar.activation(rhs[:, a:b, 1, :], rhs[:, a:b, 0, :],
                                         Square)
                    for jj in range(a, b):
                        j = js + jj
                        nc.tensor.matmul(
                            psum_sq[:], M8[:, jj, :], rhs[:, jj, :, :],
                            start=(j == 0), stop=(j == j_full - 1))

            out_t = constp.tile([C, 2, C], f32)
            nc.vector.tensor_copy(out_t[:], psum_sq[:])
            nc.sync.dma_start(sq_d[:], out_t[:])

    nc.compile()
    return nc


def _get_nc():
    key = CHUNKS
    if key not in _NC_CACHE:
        _NC_CACHE[key] = build_nc(key)
    return _NC_CACHE[key]


def kernel(column_logits, column_assignments, valid_mask):
    global LAST_RESULT
    from concourse.bass_utils import run_bass_kernel_spmd

    logits = np.asarray(column_logits, dtype=np.float32).reshape(N_TOK, C)
    seg = np.asarray(column_assignments).reshape(N_TOK).astype(np.int64)
    w = np.asarray(valid_mask).reshape(N_TOK).astype(bool)

    fp8np = ml_dtypes.float8_e4m3
    M8_full = np.zeros((N_TOK, C), dtype=fp8np)
    M8_full[np.arange(N_TOK)[w], seg[w]] = fp8np(1.0)   # w folded into M

    in_maps = []
    for i in range(NCORES):
        sl = slice(i * TOK_PER_CORE, (i + 1) * TOK_PER_CORE)
        in_maps.append({
            "logits": np.ascontiguousarray(logits[sl]),
            "m8": np.ascontiguousarray(M8_full[sl]),
        })

    nc = _get_nc()
    res = run_bass_kernel_spmd(nc, in_maps, list(range(NCORES)), trace=TRACE,
                               tmpdir=TRACE_TMPDIR)
    LAST_RESULT = res

    SQ = np.zeros((C, 2, C), np.float64)
    for rm in res.results:
        SQ += np.asarray(rm["sq_out"], dtype=np.float64)
    S = SQ[:, 0, :]
    Q = SQ[:, 1, :]

    n = np.bincount(seg[w], minlength=C).astype(np.float64)
    n_safe = np.maximum(n, 1.0)
    ssd_sum = Q.sum(axis=1) - (S * S).sum(axis=1) / n_safe
    col_var = ssd_sum / (n_safe * C)
    has_multi = n > 1.0
    count = has_multi.sum()
    total = np.where(has_multi, col_var, 0.0).sum()
    loss = total / max(count, 1.0) if count > 0 else 0.0
    return np.asarray(loss, dtype=np.float32)


# revision 19
# speedup vs baseline: 1.1524x; 1.0045x over previous
"""Trainium2 Bass kernel for ColumnConsistencyLoss (segment_reduce).

Problem: B=16, T=8192, C=128.
  probs = softmax(logits, -1)           # (N, C), N = B*T = 131072
  per column-id c (segment): n_c = #valid tokens, S_c = sum w*p, Q_c = sum w*p^2
  col_var_c = (sum_j Q_cj - sum_j S_cj^2 / n_safe_c) / (n_safe_c * C)
  loss = mean over columns with n_c > 1 of col_var_c

Sharding: data-parallel over tokens — each of the 8 cores processes
N/8 = 16384 tokens and produces partial per-segment accumulators
S (C x C) and Q (C x C).  The cross-core reduction of these tiny
accumulators plus the final scalar math happens on the host (exact
counts n_c are computed on host via bincount).

Device kernel per core (v3 — single streaming sweep, tokens on partitions):
  - host precomputes M = onehot(seg) * w  (fp8, exact 0/1 values)
  - DMA logits in non-uniform chunks [8,16,32,32,32,8] tiles (small first
    chunk fills the pipeline early; small last chunk shortens the tail)
  - ScalarE: E = exp(L) -> bf16 (big-FD chunk)
  - DVE:     d = rowsum(E) via 2x halving adds (bf16 2x mode) + reduce
  - DVE:     r ~= 1/d (reciprocal_approx_fast, ~51 ULP)
  - DVE:     rhs[:, :, 0, :] = E * broadcast(r)   (one chunk-level TT)
  - ScalarE: rhs[:, :, 1, :] = Square(rhs[:, :, 0, :])  (2 half-chunk ACTs)
  - PE:      psum[(c),(s,j')] += M_j^T @ rhs[:,j,:,:]  (F=256, fp32 accum)
The matmul contracts the 128 partitions (tokens); w rides in M:
  psum[c,0,:] = sum_t w*1[seg=c] * (E/d)        = S_c
  psum[c,1,:] = sum_t w*1[seg=c] * (E/d)^2      = Q_c   (w^2 = w).
"""

import numpy as np
import ml_dtypes

NCORES = 8
P = 128           # partitions
C = 128           # columns / segments
B, T = 16, 8192
N_TOK = B * T
TOK_PER_CORE = N_TOK // NCORES   # 16384
J_FULL = TOK_PER_CORE // P       # 128 free-columns (token tiles) per core
CHUNKS = (4, 12, 24, 28, 28, 24, 8)  # token tiles per DMA/compute chunk

TRACE = False          # set True (e.g. from test.py) to capture NTFF profile
TRACE_TMPDIR = None    # where trace/NEFF artifacts land when TRACE is set
LAST_RESULT = None     # BassKernelResults of the last run (for profiling)

_NC_CACHE = {}


def build_nc(chunks=CHUNKS):
    """Build + compile the Bass program (SPMD; same NEFF on all cores)."""
    from concourse import bacc, mybir
    import concourse.tile as tile

    f32 = mybir.dt.float32
    bf16 = mybir.dt.bfloat16
    fp8 = mybir.dt.float8e4
    Exp = mybir.ActivationFunctionType.Exp
    Square = mybir.ActivationFunctionType.Square
    Alu = mybir.AluOpType

    j_full = sum(chunks)
    tok = j_full * P
    H = C // 2   # 64
    Q4 = C // 4  # 32

    nc = bacc.Bacc("TRN2", target_bir_lowering=False, debug=False,
                   enable_asserts=False)

    lg_d = nc.dram_tensor("logits", [tok, C], f32, kind="ExternalInput")
    m_d = nc.dram_tensor("m8", [tok, C], fp8, kind="ExternalInput")
    sq_d = nc.dram_tensor("sq_out", [C, 2, C], f32, kind="ExternalOutput")

    with tile.TileContext(nc) as tc:
        with (
            tc.tile_pool(name="const", bufs=1) as constp,
            tc.tile_pool(name="ld", bufs=4) as ldp,
            tc.tile_pool(name="big", bufs=4) as bigp,
            tc.tile_pool(name="small", bufs=4) as smallp,
            tc.tile_pool(name="psum", bufs=1, space="PSUM") as psump,
        ):
            psum_sq = psump.tile([C, 2, C], f32)

            # DRAM views: (p, j, c) with token t = p*j_full + j
            lg_ap = lg_d[:].rearrange("(p j) c -> p j c", j=j_full)
            m_ap = m_d[:].rearrange("(p j) c -> p j c", j=j_full)

            nchunk = len(chunks)
            offs = [sum(chunks[:k]) for k in range(nchunk)]
            Ls = [None] * nchunk
            Ms = [None] * nchunk
            Es = [None] * nchunk

            def emit_load(k):
                cj = chunks[k]
                L = ldp.tile([P, cj, C], f32, tag="L")
                nc.sync.dma_start(L[:], lg_ap[:, offs[k]:offs[k] + cj, :])
                M8 = ldp.tile([P, cj, C], fp8, tag="M8")
                # scalar-issued DMA rides the second HWDGE ring, so the
                # one-hot load doesn't queue behind the logits stream
                nc.scalar.dma_start(M8[:], m_ap[:, offs[k]:offs[k] + cj, :])
                Ls[k], Ms[k] = L, M8

            def halves(cj):
                # sub-ranges to process at half-chunk granularity
                if cj >= 16:
                    return [(0, cj // 2), (cj // 2, cj)]
                return [(0, cj)]

            def emit_exp(k):
                cj = chunks[k]
                E = bigp.tile([P, cj, C], bf16, tag="E")
                for a, b in halves(cj):
                    nc.scalar.activation(E[:, a:b, :], Ls[k][:, a:b, :], Exp)
                Es[k] = E

            emit_load(0)
            emit_load(1)
            emit_exp(0)
            for k, cj in enumerate(chunks):
                if k + 2 < nchunk:
                    emit_load(k + 2)
                E, M8, js = Es[k], Ms[k], offs[k]
                rhs = bigp.tile([P, cj, 2, C], bf16, tag="rhs")
                sub = halves(cj)

                h1 = bigp.tile([P, cj, H], bf16, tag="h1")
                h2 = bigp.tile([P, cj, Q4], bf16, tag="h2")
                d = smallp.tile([P, cj], f32, tag="d")
                r = smallp.tile([P, cj], f32, tag="r")
                for si, (a, b) in enumerate(sub):
                    w = b - a
                    # d = rowsum(E): two bf16 2x halving adds + 1x reduce
                    nc.vector.tensor_tensor(h1[:, a:b, :], E[:, a:b, 0:H],
                                            E[:, a:b, H:C], op=Alu.add)
                    nc.vector.tensor_tensor(h2[:, a:b, :], h1[:, a:b, 0:Q4],
                                            h1[:, a:b, Q4:H], op=Alu.add)
                    nc.vector.tensor_reduce(d[:, a:b], h2[:, a:b, :],
                                            axis=mybir.AxisListType.X,
                                            op=Alu.add)
                    nc.vector.reciprocal_approx_fast(r[:, a:b], d[:, a:b])
                    # normalized probs: TT with broadcast in1
                    nc.vector.tensor_tensor(
                        rhs[:, a:b, 0, :], E[:, a:b, :],
                        r[:, a:b, None].to_broadcast([P, w, C]),
                        op=Alu.mult)
                    # next chunk's exp between this chunk's halves so the
                    # next DVE chain is never starved behind scalar work
                    if si == 0 and k + 1 < nchunk:
                        emit_exp(k + 1)
                    nc.scalar.activation(rhs[:, a:b, 1, :], rhs[:, a:b, 0, :],
                                         Square)
                    for jj in range(a, b):
                        j = js + jj
                        nc.tensor.matmul(
                            psum_sq[:], M8[:, jj, :], rhs[:, jj, :, :],
                            start=(j == 0), stop=(j == j_full - 1))

            out_t = constp.tile([C, 2, C], f32)
            nc.vector.tensor_copy(out_t[:], psum_sq[:])
            nc.sync.dma_start(sq_d[:], out_t[:])

    nc.compile()
    return nc


def _get_nc():
    key = CHUNKS
    if key not in _NC_CACHE:
        _NC_CACHE[key] = build_nc(key)
    return _NC_CACHE[key]


def kernel(column_logits, column_assignments, valid_mask):
    global LAST_RESULT
    from concourse.bass_utils import run_bass_kernel_spmd

    logits = np.asarray(column_logits, dtype=np.float32).reshape(N_TOK, C)
    seg = np.asarray(column_assignments).reshape(N_TOK).astype(np.int64)
    w = np.asarray(valid_mask).reshape(N_TOK).astype(bool)

    fp8np = ml_dtypes.float8_e4m3
    M8_full = np.zeros((N_TOK, C), dtype=fp8np)
    M8_full[np.arange(N_TOK)[w], seg[w]] = fp8np(1.0)   # w folded into M

    in_maps = []
    for i in range(NCORES):
        sl = slice(i * TOK_PER_CORE, (i + 1) * TOK_PER_CORE)
        in_maps.append({
            "logits": np.ascontiguousarray(logits[sl]),
            "m8": np.ascontiguousarray(M8_full[sl]),
        })

    nc = _get_nc()
    res = run_bass_kernel_spmd(nc, in_maps, list(range(NCORES)), trace=TRACE,
                               tmpdir=TRACE_TMPDIR)
    LAST_RESULT = res

    SQ = np.zeros((C, 2, C), np.float64)
    for rm in res.results:
        SQ += np.asarray(rm["sq_out"], dtype=np.float64)
    S = SQ[:, 0, :]
    Q = SQ[:, 1, :]

    n = np.bincount(seg[w], minlength=C).astype(np.float64)
    n_safe = np.maximum(n, 1.0)
    ssd_sum = Q.sum(axis=1) - (S * S).sum(axis=1) / n_safe
    col_var = ssd_sum / (n_safe * C)
    has_multi = n > 1.0
    count = has_multi.sum()
    total = np.where(has_multi, col_var, 0.0).sum()
    loss = total / max(count, 1.0) if count > 0 else 0.0
    return np.asarray(loss, dtype=np.float32)
